# revision 8
# baseline (speedup 1.0000x reference)
"""CorrespondenceNet retrieval kernel for 8 Trainium2 NeuronCores.

Math (reference):
  xn = x / ||x[:,c]||,  yn = y / ||y[:,c]||          (per-channel L2 over pixels)
  corr = xn @ yn.T                                    [N, N]
  attn = softmax(corr / tau, axis=0)                  (column softmax)
  warped[k]   = sum_j attn[k, j] * yn[j, k],  k < C
  conf[i]     = max_j corr[i, j]

Sharding: columns of corr (rows of y) are sharded across 8 cores. Each core
holds the full x^T (replicated) and a 1024-row slice of y, computes corr
tiles laid out [j_partition, i_free] so that:
  - column-softmax sums s_j are a free-dim reduction fused into the ACT exp
    pass (activation accum_out),
  - per-i max (confidence) is a running elementwise DVE max over j-blocks
    followed by a GPSIMD partition_all_reduce,
  - conf = tau * ln(max_j exp(corr/tau)) recovers the row max exactly
    (monotone transform), so no max-subtraction pass over corr is needed.
Softmax is computed without max subtraction: |corr| <= max_i||xn_i|| *
max_j||yn_j|| ~ 0.05, so logits are bounded by ~5 and exp() cannot overflow
even in fp16.

Host combines: elementwise max over cores (conf), sum over cores (warped).
"""

import sys

sys.path.insert(0, "/opt/trn_rl_repo")

import numpy as np

import concourse.bacc as bacc
import concourse.bass as bass
import concourse.tile as tile
from concourse import bass_isa, library_config, mybir
from concourse.bass_utils import run_bass_kernel_spmd

TAU = 0.01
N, C, M = 8192, 256, 8  # pixels, channels, cores

F16 = mybir.dt.float16
F32 = mybir.dt.float32


def build_kernel(n=N, c=C, m=M, chunk=2048, stage="full"):
    """Build the per-core Bass program (SPMD; per-core data differs)."""
    JS = n // m  # j-slice width per core
    JB = JS // 128  # j partition-blocks per core
    CH = min(chunk, n)  # i-chunk width (ACT op width)
    NCH = n // CH
    SUB = CH // 512  # matmul sub-slices per chunk
    KB = c // 128  # contraction blocks
    Q = CH // 128  # conf columns per chunk
    assert n % CH == 0 and CH % 512 == 0 and JS % 128 == 0 and c % 128 == 0
    assert CH >= c, "warped slice must sit inside chunk 0"

    nc = bacc.Bacc("TRN2", target_bir_lowering=False, debug=False, num_devices=m)

    xT = nc.dram_tensor("xT", [c, n], F16, kind="ExternalInput").ap()
    yT = nc.dram_tensor("yT", [c, n], F16, kind="ExternalInput").ap()
    ydT = nc.dram_tensor("ydT", [c, JS], F16, kind="ExternalInput").ap()
    yd = nc.dram_tensor("yd", [JS, c], F16, kind="ExternalInput").ap()
    conf_out = nc.dram_tensor("conf", [128, n // 128], F32, kind="ExternalOutput").ap()
    warp_out = nc.dram_tensor("warped", [1, c], F32, kind="ExternalOutput").ap()

    with tile.TileContext(nc) as tc:
        with (
            tc.tile_pool(name="big", bufs=1) as big,
            tc.tile_pool(name="persist", bufs=1) as persist,
            tc.tile_pool(name="expb", bufs=3) as expb,
            tc.tile_pool(name="cmax", bufs=NCH) as cmaxp,
            tc.tile_pool(name="cred", bufs=2) as credp,
            tc.tile_pool(name="esl", bufs=JB) as eslp,
            tc.tile_pool(name="ydp", bufs=JB) as ydp,
            tc.tile_pool(name="small", bufs=1) as small,
        ):
            # ---- load inputs -------------------------------------------------
            xTt = []  # c-blocks of x^T [128, n] (normalized in place below)
            yTt = []
            ydTt = []
            for b in range(KB):
                t = big.tile([128, n], F16, tag=f"xT{b}")
                nc.sync.dma_start(t[:], xT[b * 128 : (b + 1) * 128, :])
                xTt.append(t)
                t = big.tile([128, n], F16, tag=f"yT{b}")
                nc.sync.dma_start(t[:], yT[b * 128 : (b + 1) * 128, :])
                yTt.append(t)
                t = persist.tile([128, JS], F16, tag=f"ydT{b}")
                nc.sync.dma_start(t[:], ydT[b * 128 : (b + 1) * 128, :])
                ydTt.append(t)
            ydt = []
            for jb in range(JB):
                t = ydp.tile([128, c], F16, tag="yd")
                nc.sync.dma_start(t[:], yd[jb * 128 : (jb + 1) * 128, :])
                ydt.append(t)

            # ---- per-channel norms ------------------------------------------
            # n2 cols: [x blocks..., y blocks...]; rn = 1/sqrt(n2), Newton-refined.
            n2 = small.tile([128, 2 * KB], F32, tag="n2")
            scratch = big.tile([128, n], F16, tag="scratch")
            for b in range(KB):
                nc.scalar.activation(
                    scratch[:], xTt[b][:],
                    mybir.ActivationFunctionType.Square,
                    accum_out=n2[:, b : b + 1],
                )
            for b in range(KB):
                nc.scalar.activation(
                    scratch[:], yTt[b][:],
                    mybir.ActivationFunctionType.Square,
                    accum_out=n2[:, KB + b : KB + b + 1],
                )
            sq = small.tile([128, 2 * KB], F32, tag="sq")
            nc.scalar.sqrt(sq[:], n2[:])
            r0 = small.tile([128, 2 * KB], F32, tag="r0")
            nc.vector.reciprocal(r0[:], sq[:])
            # one Newton step on rsqrt: rn = r0 * (1.5 - 0.5 * n2 * r0^2)
            t1 = small.tile([128, 2 * KB], F32, tag="t1")
            nc.vector.tensor_mul(t1[:], r0[:], r0[:])
            nc.vector.tensor_mul(t1[:], t1[:], n2[:])
            nc.vector.tensor_scalar(
                t1[:], t1[:], -0.5, 1.5, mybir.AluOpType.mult, mybir.AluOpType.add
            )
            rn = small.tile([128, 2 * KB], F32, tag="rn")
            nc.vector.tensor_mul(rn[:], r0[:], t1[:])

            # ---- normalize operands in place --------------------------------
            for b in range(KB):
                nc.vector.tensor_scalar(
                    xTt[b][:], xTt[b][:], rn[:, b : b + 1], None, mybir.AluOpType.mult
                )
                nc.vector.tensor_scalar(
                    ydTt[b][:], ydTt[b][:], rn[:, KB + b : KB + b + 1], None,
                    mybir.AluOpType.mult,
                )

            # ---- main stream: corr -> exp/s_j/cmax ---------------------------
            s_all = persist.tile([128, JB * NCH], F32, tag="s_all")
            cmax = [cmaxp.tile([128, CH], F16, tag="cmax", name=f"cmax{i}") for i in range(NCH)]
            esl = [eslp.tile([128, c], F16, tag="esl", name=f"esl{i}") for i in range(JB)]
            confM = persist.tile([128, n // 128], F16, tag="confM")

            with tc.tile_pool(name="psum", bufs=2, space="PSUM") as psum_pool:
                for ch in range(NCH):
                    for jb in range(JB):
                        pt = psum_pool.tile([128, CH], F32, tag="corr")
                        for s in range(SUB):
                            off = ch * CH + s * 512
                            for k in range(KB):
                                nc.tensor.matmul(
                                    pt[:, s * 512 : (s + 1) * 512],
                                    ydTt[k][:, jb * 128 : (jb + 1) * 128],
                                    xTt[k][:, off : off + 512],
                                    start=(k == 0),
                                    stop=(k == KB - 1),
                                )
                        e = expb.tile([128, CH], F16, tag="e")
                        nc.scalar.activation(
                            e[:], pt[:],
                            mybir.ActivationFunctionType.Exp,
                            scale=1.0 / TAU,
                            accum_out=s_all[:, jb * NCH + ch : jb * NCH + ch + 1],
                        )
                        if ch == 0:
                            nc.vector.tensor_copy(esl[jb][:], e[:, 0:c])
                        if jb == 0:
                            nc.vector.tensor_copy(cmax[ch][:], e[:])
                        else:
                            nc.vector.tensor_max(cmax[ch][:], cmax[ch][:], e[:])
                    if stage in ("full", "nowarp"):
                        cr = credp.tile([128, CH], F16, tag="cred")
                        nc.gpsimd.partition_all_reduce(
                            cr[:], cmax[ch][:], 128, bass_isa.ReduceOp.max
                        )
                        nc.sync.dma_start(confM[:, ch * Q : (ch + 1) * Q], cr[0:1, :])
                    else:
                        nc.vector.tensor_copy(confM[:, ch * Q : (ch + 1) * Q], cmax[ch][:, 0 : Q])

            # ---- confidence: tau * ln(max exp) ------------------------------
            lnM = persist.tile([128, n // 128], F32, tag="lnM")
            nc.scalar.activation(lnM[:], confM[:], mybir.ActivationFunctionType.Ln)
            conf_f = persist.tile([128, n // 128], F32, tag="conf_f")
            nc.vector.tensor_scalar(
                conf_f[:], lnM[:], TAU, None, mybir.AluOpType.mult
            )
            nc.sync.dma_start(conf_out[:], conf_f[:])

            # ---- warped: sum_j (exp * (1/s_j)) * y_raw, then * rn_y[k] ------
            if stage in ("full", "noconf"):
                s8 = small.tile([128, JB], F32, tag="s8")
                for jb in range(JB):
                    nc.vector.reduce_sum(
                        s8[:, jb : jb + 1],
                        s_all[:, jb * NCH : (jb + 1) * NCH],
                        axis=mybir.AxisListType.X,
                    )
                r8 = small.tile([128, JB], F32, tag="r8")
                nc.vector.reciprocal(r8[:], s8[:])

                ones = small.tile([128, 1], F16, tag="ones")
                nc.vector.memset(ones[:], 1.0)
                with tc.tile_pool(name="psw", bufs=1, space="PSUM") as psw:
                    pw = psw.tile([1, c], F32, tag="pw")
                    for jb in range(JB):
                        w = small.tile([128, c], F16, tag="wtile")
                        nc.vector.scalar_tensor_tensor(
                            w[:], esl[jb][:], r8[:, jb : jb + 1], ydt[jb][:],
                            mybir.AluOpType.mult, mybir.AluOpType.mult,
                        )
                        nc.tensor.matmul(
                            pw[:], ones[:], w[:], start=(jb == 0), stop=(jb == JB - 1)
                        )
                    # rn_y as a [1, c] row: DMA partition column -> free row
                    rny_t = small.tile([1, c], F32, tag="rny_t")
                    for b in range(KB):
                        nc.sync.dma_start(
                            rny_t[0:1, b * 128 : (b + 1) * 128],
                            rn[:, KB + b : KB + b + 1],
                        )
                    wsb = small.tile([1, c], F32, tag="wsb")
                    nc.vector.tensor_mul(wsb[:], pw[:], rny_t[:])
                    nc.sync.dma_start(warp_out[:], wsb[:])

    nc.compile()
    return nc


def _shard_inputs(x, y, n, c, m):
    """Host-side shard/layout: transpose + fp16 cast only (no module math)."""
    js = n // m
    xT = np.ascontiguousarray(x.T.astype(np.float16))
    yT = np.ascontiguousarray(y.T.astype(np.float16))
    in_maps = []
    for d in range(m):
        in_maps.append(
            {
                "xT": xT,
                "yT": yT,
                "ydT": np.ascontiguousarray(yT[:, d * js : (d + 1) * js]),
                "yd": np.ascontiguousarray(y[d * js : (d + 1) * js, :].astype(np.float16)),
            }
        )
    return in_maps


def _combine_outputs(results, n, c, m, chunk=2048):
    ch = min(chunk, n)
    nch = n // ch
    q = ch // 128
    confs = []
    warps = []
    for d in range(m):
        cm = results[d]["conf"]  # [128, n//128]
        confs.append(cm.reshape(128, nch, q).transpose(1, 0, 2).reshape(n))
        warps.append(results[d]["warped"].reshape(c))
    conf = np.maximum.reduce(confs).astype(np.float32)
    warped = np.sum(np.stack(warps, 0), axis=0, dtype=np.float32)
    return warped, conf


_CACHED = {}


def kernel(x_feature, y_feature):
    x = np.asarray(x_feature, dtype=np.float32)
    y = np.asarray(y_feature, dtype=np.float32)
    assert x.shape == (N, C) and y.shape == (N, C)
    if "nc" not in _CACHED:
        _CACHED["nc"] = build_kernel(N, C, M)
    nc = _CACHED["nc"]
    in_maps = _shard_inputs(x, y, N, C, M)
    res = run_bass_kernel_spmd(nc, in_maps, core_ids=list(range(M)))
    warped, conf = _combine_outputs(res.results, N, C, M)
    return warped, conf


if __name__ == "__main__":
    rng = np.random.default_rng(0)
    x = rng.normal(size=(N, C)).astype(np.float32)
    y = rng.normal(size=(N, C)).astype(np.float32)
    w, cf = kernel(x, y)
    print("warped", w[:4], "conf", cf[:4])
